# revision 1
# baseline (speedup 1.0000x reference)
"""MoE layer (8 experts, top-2) on 8 TRN2 NeuronCores, expert-parallel.

Strategy (sparse dispatch, per the sharding hint):
  - Core m owns expert m (w1[m], w2[m], b1[m], b2[m]).
  - Host computes top-2 expert ids per token (fp32 router, dispatch only)
    and "all-to-all"s: each core receives only the tokens routed to its
    expert, gathered as X_c^T [H, C] (C = max expert load, rounded to 128).
  - On device, each core re-runs the router (fp32 matmul on PE) over its
    gathered tokens and derives ITS OWN expert's combine weight per token
    purely elementwise:
        w_e(t) = exp(l_e - m1) / (1 + exp(m2 - m1))  if l_e >= m2 else 0
    (equals softmax-top2-renormalize of the reference).
  - FFN in bf16 (f32 PSUM accumulate): h1 = gelu(x @ w1 + b1) in [F, C]
    layout; y = (h1^T @ w2 + b2) * w with tokens on partitions -> yc [C, H].
  - Host scatter-adds each core's weighted outputs back to token order.
"""

from contextlib import ExitStack

import ml_dtypes
import numpy as np

P = 128
B, S, H, F, E = 2, 2048, 1024, 4096, 8
T = B * S            # 4096 tokens
KH = H // P          # 8   k-subtiles over H
KF = F // P          # 32  k-subtiles over F

_CACHE = {}


def _chunks(C):
    out = []
    t0 = 0
    while t0 < C:
        size = min(512, C - t0)
        out.append((t0, size))
        t0 += size
    return out


def _build_nc(C, reps=1):
    import concourse.mybir as mybir
    import concourse.tile as tile
    from concourse import bacc

    dt = mybir.dt
    AF = mybir.ActivationFunctionType
    ALU = mybir.AluOpType
    AX = mybir.AxisListType

    TTc = C // P  # token tiles

    nc = bacc.Bacc(
        "TRN2", target_bir_lowering=False, debug=False, num_devices=E)

    xct32 = nc.declare_dram_parameter("xct32", [H, C], dt.float32, isOutput=False)
    xctb = nc.declare_dram_parameter("xctb", [H, C], dt.bfloat16, isOutput=False)
    rw = nc.declare_dram_parameter("rw", [H, E], dt.float32, isOutput=False)
    rbb = nc.declare_dram_parameter("rbb", [P, E], dt.float32, isOutput=False)
    selb = nc.declare_dram_parameter("selb", [P, E], dt.float32, isOutput=False)
    w1d = nc.declare_dram_parameter("w1d", [H, F], dt.bfloat16, isOutput=False)
    w2d = nc.declare_dram_parameter("w2d", [F, H], dt.bfloat16, isOutput=False)
    b1d = nc.declare_dram_parameter("b1d", [P, KF], dt.float32, isOutput=False)
    b2b = nc.declare_dram_parameter("b2b", [P, H], dt.float32, isOutput=False)
    yc = nc.declare_dram_parameter("yc", [C, H], dt.float32, isOutput=True)

    xct32_r = xct32.rearrange("(k p) t -> p k t", p=P)
    xctb_r = xctb.rearrange("(k p) t -> p k t", p=P)
    rw_r = rw.rearrange("(k p) e -> p k e", p=P)
    w1_r = w1d.rearrange("(k p) f -> p k f", p=P)
    w2_r = w2d.rearrange("(k p) h -> p k h", p=P)

    with ExitStack() as ctx:
        tc = ctx.enter_context(tile.TileContext(nc))
        const = ctx.enter_context(tc.tile_pool(name="const", bufs=1))
        xrpool = ctx.enter_context(tc.tile_pool(name="xr", bufs=2))
        rpool = ctx.enter_context(tc.tile_pool(name="rtmp", bufs=3))
        rpsum = ctx.enter_context(tc.tile_pool(name="rpsum", bufs=1, space="PSUM"))
        xpool = ctx.enter_context(tc.tile_pool(name="xc", bufs=2))
        h1pool = ctx.enter_context(tc.tile_pool(name="h1", bufs=1))
        p1pool = ctx.enter_context(tc.tile_pool(name="p1", bufs=5, space="PSUM"))
        p2pool = ctx.enter_context(tc.tile_pool(name="p2", bufs=2, space="PSUM"))
        opool = ctx.enter_context(tc.tile_pool(name="ob", bufs=8))

        # Small constants first so nothing queues behind the weight stacks.
        # (b2b is 0.5MB and not needed until the first output stage ~70us in,
        # so it loads after the weight stream instead.)
        rbb_s = const.tile([P, E], dt.float32)
        nc.sync.dma_start(rbb_s[:], rbb[:])
        selb_s = const.tile([P, E], dt.float32)
        nc.sync.dma_start(selb_s[:], selb[:])
        b1_s = const.tile([P, KF], dt.float32)
        nc.sync.dma_start(b1_s[:], b1d[:])
        rw_s = const.tile([P, KH, E], dt.float32)
        b2b_s = const.tile([P, H], dt.float32)
        wmat = const.tile([P, TTc], dt.float32)

        chunks = _chunks(C)

        def load_xc(t0, csz):
            xc = xpool.tile([P, KH, 512], dt.bfloat16, name="xc")[:, :, :csz]
            for k in range(KH):
                nc.sync.dma_start(xc[:, k], xctb_r[:, k, t0:t0 + csz])
            return xc

        # Startup: interleave chunk-0 activations with w1's first f-chunk
        # per k so the first matmul group is runnable after ~2MB of DMA.
        # Then w1 f-chunk-major with w2 k-slices interleaved at a ratio
        # that keeps DMA just ahead of PE's w1 consumption, so w2 is
        # resident before chunk-0 matmul2 starts (~70us in).
        w1_s = const.tile([P, KH, F], dt.bfloat16)
        w2_s = const.tile([P, KF, H], dt.bfloat16)
        xc0 = xpool.tile([P, KH, 512], dt.bfloat16, name="xc")[:, :, :chunks[0][1]]
        for k in range(KH):
            nc.sync.dma_start(xc0[:, k], xctb_r[:, k, 0:chunks[0][1]])
            nc.sync.dma_start(w1_s[:, k, 0:512], w1_r[:, k, 0:512])
        w2_next = 0
        for fc in range(1, F // 512):
            for k in range(KH):
                nc.sync.dma_start(
                    w1_s[:, k, fc * 512:(fc + 1) * 512],
                    w1_r[:, k, fc * 512:(fc + 1) * 512])
            share = 0 if fc < 2 else (5 if fc < 7 else KF - w2_next)
            for k in range(w2_next, w2_next + share):
                nc.sync.dma_start(w2_s[:, k], w2_r[:, k])
            w2_next += share
            if fc == 4:
                nc.sync.dma_start(rw_s[:], rw_r)
        nc.sync.dma_start(b2b_s[:], b2b[:])

        def emit_mm1(xc, csz):
            h1 = h1pool.tile([P, KF, 512], dt.bfloat16, name="h1")[:, :, :csz]
            for f in range(KF):
                ps1 = p1pool.tile([P, 512], dt.float32, name="ps1")[:, :csz]
                for k in range(KH):
                    nc.tensor.matmul(
                        ps1[:], w1_s[:, k, f * P:(f + 1) * P], xc[:, k],
                        start=(k == 0), stop=(k == KH - 1),
                    )
                nc.scalar.activation(h1[:, f], ps1[:], AF.Gelu, bias=b1_s[:, f:f + 1])
            return h1

        def emit_mm2(h1, t0, csz, tail_split=False):
            for ct in range(csz // P):
                gt = t0 // P + ct
                for hh in range(H // 512):
                    last = tail_split and ct == csz // P - 1 and hh == H // 512 - 1
                    # The very last group splits in two halves so its output
                    # pipeline (DVE + DMA) overlaps the second half's matmuls
                    # instead of running serially after PE finishes.
                    for (o0, wid) in ([(0, 256), (256, 128), (384, 64), (448, 64)] if last else [(0, 512)]):
                        ps2 = p2pool.tile([P, 512], dt.float32, name="ps2")[:, :wid]
                        for k in range(KF):
                            nc.tensor.matmul(
                                ps2[:], h1[:, k, ct * P:(ct + 1) * P],
                                w2_s[:, k, hh * 512 + o0:hh * 512 + o0 + wid],
                                start=(k == 0), stop=(k == KF - 1),
                            )
                        ob = opool.tile([P, 512], dt.float32, name="ob")[:, :wid]
                        nc.vector.tensor_tensor(
                            ob[:], ps2[:],
                            b2b_s[:, hh * 512 + o0:hh * 512 + o0 + wid], ALU.add)
                        nc.vector.tensor_scalar_mul(ob[:], ob[:], wmat[:, gt:gt + 1])
                        nc.sync.dma_start(
                            yc[gt * P:(gt + 1) * P,
                               hh * 512 + o0:hh * 512 + o0 + wid], ob[:])

        for _rep in range(reps):
            # Chunk-0 first FFN matmul overlaps the router's DMAs.
            h1_0 = emit_mm1(xc0, chunks[0][1])

            # ---- Router: combine weight of MY expert for my gathered tokens ----
            for tt in range(TTc):
                xt_t = xrpool.tile([P, KH, P], dt.float32)
                nc.sync.dma_start(xt_t[:], xct32_r[:, :, tt * P:(tt + 1) * P])
                lg = rpsum.tile([P, E], dt.float32)
                for k in range(KH):
                    nc.tensor.matmul(
                        lg[:], xt_t[:, k], rw_s[:, k],
                        start=(k == 0), stop=(k == KH - 1),
                    )
                l = rpool.tile([P, E], dt.float32)
                nc.vector.tensor_tensor(l[:], lg[:], rbb_s[:], ALU.add)
                m1 = rpool.tile([P, 1], dt.float32)
                nc.vector.reduce_max(m1[:], l[:], axis=AX.X)
                nm1 = rpool.tile([P, 1], dt.float32)
                nc.vector.tensor_scalar_mul(nm1[:], m1[:], -1.0)
                ismax = rpool.tile([P, E], dt.float32)
                nc.vector.tensor_tensor(
                    ismax[:], l[:], m1[:].to_broadcast((P, E)), ALU.is_equal)
                pen = rpool.tile([P, E], dt.float32)
                nc.vector.tensor_scalar_mul(pen[:], ismax[:], 1e30)
                lmask = rpool.tile([P, E], dt.float32)
                nc.vector.tensor_tensor(lmask[:], l[:], pen[:], ALU.subtract)
                m2 = rpool.tile([P, 1], dt.float32)
                nc.vector.reduce_max(m2[:], lmask[:], axis=AX.X)
                lsel = rpool.tile([P, E], dt.float32)
                nc.vector.tensor_tensor(lsel[:], l[:], selb_s[:], ALU.mult)
                lmine = rpool.tile([P, 1], dt.float32)
                nc.vector.reduce_sum(lmine[:], lsel[:], axis=AX.X)
                ge = rpool.tile([P, 1], dt.float32)
                nc.vector.tensor_tensor(ge[:], lmine[:], m2[:], ALU.is_ge)
                e1 = rpool.tile([P, 1], dt.float32)
                nc.scalar.activation(e1[:], lmine[:], AF.Exp, bias=nm1[:])
                e2 = rpool.tile([P, 1], dt.float32)
                nc.scalar.activation(e2[:], m2[:], AF.Exp, bias=nm1[:])
                den = rpool.tile([P, 1], dt.float32)
                nc.vector.tensor_scalar_add(den[:], e2[:], 1.0)
                rec = rpool.tile([P, 1], dt.float32)
                nc.vector.reciprocal(rec[:], den[:])
                wnum = rpool.tile([P, 1], dt.float32)
                nc.vector.tensor_tensor(wnum[:], e1[:], ge[:], ALU.mult)
                nc.vector.tensor_tensor(wmat[:, tt:tt + 1], wnum[:], rec[:], ALU.mult)

            # ---- Expert FFN over gathered tokens, weighted output ----
            emit_mm2(h1_0, chunks[0][0], chunks[0][1],
                     tail_split=(len(chunks) == 1))
            for ci, (t0, csz) in enumerate(chunks[1:], start=1):
                xc = load_xc(t0, csz)
                h1 = emit_mm1(xc, csz)
                emit_mm2(h1, t0, csz, tail_split=(ci == len(chunks) - 1))
    return nc


def _get_nc(C, reps=1):
    key = (C, reps)
    if key not in _CACHE:
        nc = _build_nc(C, reps)
        nc.finalize()
        _CACHE[key] = nc
    return _CACHE[key]


def dispatch(hidden_states, router_w, router_b):
    """Host-side top-2 dispatch: per-expert token index lists + capacity."""
    x = np.asarray(hidden_states, dtype=np.float32).reshape(T, H)
    logits = x @ np.asarray(router_w, dtype=np.float32)
    logits = logits + np.asarray(router_b, dtype=np.float32)
    top2 = np.argpartition(logits, E - 2, axis=1)[:, E - 2:]  # [T, 2] unordered
    idx_lists = []
    for m in range(E):
        idx_lists.append(np.where((top2 == m).any(axis=1))[0])
    cmax = max(len(ix) for ix in idx_lists)
    C = max(P, ((cmax + P - 1) // P) * P)
    return x, idx_lists, C


def make_in_maps(hidden_states, router_w, router_b, w1, b1, w2, b2):
    bf16 = ml_dtypes.bfloat16
    x, idx_lists, C = dispatch(hidden_states, router_w, router_b)
    xt = np.ascontiguousarray(x.T)            # [H, T] f32
    xtb = xt.astype(bf16)
    rw = np.ascontiguousarray(np.asarray(router_w, dtype=np.float32))
    rbb = np.ascontiguousarray(
        np.broadcast_to(np.asarray(router_b, dtype=np.float32), (P, E)))
    w1 = np.asarray(w1, dtype=np.float32)
    w2 = np.asarray(w2, dtype=np.float32)
    b1 = np.asarray(b1, dtype=np.float32)
    b2 = np.asarray(b2, dtype=np.float32)
    in_maps = []
    for m in range(E):
        ix = idx_lists[m]
        pad = np.zeros(C, dtype=np.int64)
        pad[:len(ix)] = ix
        sel = np.zeros((P, E), dtype=np.float32)
        sel[:, m] = 1.0
        in_maps.append({
            "xct32": np.ascontiguousarray(xt[:, pad]),
            "xctb": np.ascontiguousarray(xtb[:, pad]),
            "rw": rw,
            "rbb": rbb,
            "selb": sel,
            "w1d": np.ascontiguousarray(w1[m].astype(bf16)),
            "w2d": np.ascontiguousarray(w2[m].astype(bf16)),
            "b1d": np.ascontiguousarray(b1[m].reshape(KF, P).T),
            "b2b": np.ascontiguousarray(np.broadcast_to(b2[m], (P, H))),
        })
    return in_maps, idx_lists, C


def run_device(in_maps, C):
    from concourse.bass_utils import run_bass_kernel_spmd

    nc = _get_nc(C)
    res = run_bass_kernel_spmd(nc, in_maps, core_ids=list(range(E)))
    return res.results


def kernel(hidden_states, router_w, router_b, w1, b1, w2, b2):
    in_maps, idx_lists, C = make_in_maps(
        hidden_states, router_w, router_b, w1, b1, w2, b2)
    # One retry guards against a rare transient execution glitch observed on
    # the very first load of a freshly compiled NEFF (garbage ~1e35 values);
    # a healthy output has absmax of a few units.
    last_err = None
    for attempt in range(3):
        try:
            results = run_device(in_maps, C)
        except Exception as e:  # transient NRT/axon failures observed
            last_err = e
            import time as _time
            _time.sleep(10)
            continue
        acc = np.zeros((T, H), dtype=np.float32)
        for m in range(E):
            ix = idx_lists[m]
            acc[ix] += np.asarray(results[m]["yc"], dtype=np.float32)[:len(ix)]
        if np.isfinite(acc).all() and np.abs(acc).max() < 1e4:
            return acc.reshape(B, S, H)
    if last_err is not None:
        raise last_err
    return acc.reshape(B, S, H)



# revision 3
# speedup vs baseline: 1.3055x; 1.3055x over previous
"""MoE layer (8 experts, top-2) on 8 TRN2 NeuronCores, expert-parallel.

Strategy (sparse dispatch per the sharding hint, fp8 DoubleRow FFN):
  - Core m owns expert m (w1[m], w2[m], b1[m], b2[m]).
  - Host computes the router exactly (fp32 numpy), does the top-2
    dispatch ("all-to-all": each core receives only the tokens routed to
    its expert) and ships the per-token combine weight, so the device
    does only the expert FFN.
  - FFN runs on the PE in fp8-e4m3 DoubleRow mode (two 128-row k-tiles
    per instruction) with full error compensation: every operand is
    split into hi + lo fp8 parts (lo = residual of the hi quantization)
    and each matmul accumulates three passes in one PSUM group:
        hi@hi + lo@hi + hi@lo    (the lo@lo term is negligible)
    Weight tensors are pre-scaled by 256 on the host so every pass lands
    at the same power-of-2 scale; the 1/256 is folded into the gelu
    scale (mm1) and the combine weight (mm2).
  - h = gelu(x @ w1 + b1) is written twice by the scalar engine (fp8 hi
    + f32), the DVE derives the fp8 lo residual.
  - Host scatter-adds each core's weighted outputs back to token order.
"""

from contextlib import ExitStack

import ml_dtypes
import numpy as np

P = 128
B, S, H, F, E = 2, 2048, 1024, 4096, 8
T = B * S            # 4096 tokens
J = H // 256         # 4  mm1 k-tile pairs
G = F // 256         # 16 mm2 k-tile pairs
FB = F // P          # 32 mm1 output f-blocks
HB = H // 256        # 4  mm2 output h-blocks
CK = 256             # token chunk

fp8 = ml_dtypes.float8_e4m3fn

_CACHE = {}


def _build_nc(C):
    import concourse.mybir as mybir
    import concourse.tile as tile
    from concourse import bacc

    dt = mybir.dt
    AF = mybir.ActivationFunctionType
    ALU = mybir.AluOpType
    PM = mybir.MatmulPerfMode

    NC = (C + CK - 1) // CK          # chunks (last may be 128 tokens)
    sizes = [min(CK, C - c * CK) for c in range(NC)]
    Cx = NC * CK                     # x layout padded to full chunks
    TTS = C // P                     # token tiles

    nc = bacc.Bacc(
        "TRN2", target_bir_lowering=False, debug=False, num_devices=E)

    xh = nc.declare_dram_parameter("xh", [P, Cx * 8], dt.float8e4, isOutput=False)
    xl = nc.declare_dram_parameter("xl", [P, Cx * 8], dt.float8e4, isOutput=False)
    w1h = nc.declare_dram_parameter("w1h", [P, FB * 8 * P], dt.float8e4, isOutput=False)
    w1l = nc.declare_dram_parameter("w1l", [P, FB * 8 * P], dt.float8e4, isOutput=False)
    w2h = nc.declare_dram_parameter("w2h", [P, HB * G * 512], dt.float8e4, isOutput=False)
    w2l = nc.declare_dram_parameter("w2l", [P, HB * G * 512], dt.float8e4, isOutput=False)
    b1d = nc.declare_dram_parameter("b1d", [P, FB], dt.float32, isOutput=False)
    b2w = nc.declare_dram_parameter("b2w", [P, H], dt.float32, isOutput=False)
    wdv = nc.declare_dram_parameter("wdv", [P, TTS], dt.float32, isOutput=False)
    yc = nc.declare_dram_parameter("yc", [C, H], dt.float32, isOutput=True)

    xh_r = xh.rearrange("p (c j i t) -> p c j i t", c=NC, j=J, i=2)
    xl_r = xl.rearrange("p (c j i t) -> p c j i t", c=NC, j=J, i=2)
    w1h_r = w1h.rearrange("p (fb j i f) -> p fb j i f", fb=FB, j=J, i=2)
    w1l_r = w1l.rearrange("p (fb j i f) -> p fb j i f", fb=FB, j=J, i=2)
    w2h_r = w2h.rearrange("p (hb g i h) -> p hb g i h", hb=HB, g=G, i=2)
    w2l_r = w2l.rearrange("p (hb g i h) -> p hb g i h", hb=HB, g=G, i=2)

    with ExitStack() as ctx:
        tc = ctx.enter_context(tile.TileContext(nc))
        const = ctx.enter_context(tc.tile_pool(name="const", bufs=1))
        xpool = ctx.enter_context(tc.tile_pool(name="xt", bufs=2 * NC))
        h8pool = ctx.enter_context(tc.tile_pool(name="h8", bufs=2))
        hlpool = ctx.enter_context(tc.tile_pool(name="hl", bufs=2))
        gpool = ctx.enter_context(tc.tile_pool(name="g32", bufs=3))
        p1pool = ctx.enter_context(tc.tile_pool(name="p1", bufs=4, space="PSUM"))
        p2pool = ctx.enter_context(tc.tile_pool(name="p2", bufs=3, space="PSUM"))
        opool = ctx.enter_context(tc.tile_pool(name="ob", bufs=4))

        # ---- DMA schedule: consts, chunk-0 x, w1 (4fb slices, hi/lo
        # interleaved), x1, w2 (hb slices), remaining x chunks. ----
        b1_s = const.tile([P, FB], dt.float32)
        nc.sync.dma_start(b1_s[:], b1d[:])
        wdv_s = const.tile([P, TTS], dt.float32)
        nc.sync.dma_start(wdv_s[:], wdv[:])

        xh_s = [xpool.tile([P, J, 2, CK], dt.float8e4, name="xt") for _ in range(NC)]
        xl_s = [xpool.tile([P, J, 2, CK], dt.float8e4, name="xt") for _ in range(NC)]
        nc.sync.dma_start(xh_s[0][:], xh_r[:, 0])
        nc.sync.dma_start(xl_s[0][:], xl_r[:, 0])

        w1h_s = const.tile([P, FB, J, 2, P], dt.float8e4)
        w1l_s = const.tile([P, FB, J, 2, P], dt.float8e4)
        for s in range(8):
            sl = slice(s * 4, (s + 1) * 4)
            nc.sync.dma_start(w1h_s[:, sl], w1h_r[:, sl])
            nc.sync.dma_start(w1l_s[:, sl], w1l_r[:, sl])
            if s == 1 and NC > 1:
                nc.sync.dma_start(xh_s[1][:], xh_r[:, 1])
                nc.sync.dma_start(xl_s[1][:], xl_r[:, 1])

        b2w_s = const.tile([P, H], dt.float32)
        w2h_s = const.tile([P, HB, G, 2, 256], dt.float8e4)
        w2l_s = const.tile([P, HB, G, 2, 256], dt.float8e4)
        for hb in range(HB):
            nc.sync.dma_start(w2h_s[:, hb], w2h_r[:, hb])
            nc.sync.dma_start(w2l_s[:, hb], w2l_r[:, hb])
            if hb == 0:
                nc.sync.dma_start(b2w_s[:], b2w[:])
            c = hb + 2
            if c < NC:
                nc.sync.dma_start(xh_s[c][:], xh_r[:, c])
                nc.sync.dma_start(xl_s[c][:], xl_r[:, c])
        for c in range(HB + 2, NC):
            nc.sync.dma_start(xh_s[c][:], xh_r[:, c])
            nc.sync.dma_start(xl_s[c][:], xl_r[:, c])

        hs = [None] * NC

        def emit_mm1(c):
            csz = sizes[c]
            xht, xlt = xh_s[c], xl_s[c]
            h8 = h8pool.tile([P, G, 2, CK], dt.float8e4, name="h8")
            hl = hlpool.tile([P, G, 2, CK], dt.float8e4, name="hl")
            hs[c] = (h8, hl)
            for fb in range(FB):
                ps = p1pool.tile([P, CK], dt.float32, name="p1")[:, :csz]
                for j in range(J):
                    nc.tensor.matmul(
                        ps[:], w1h_s[:, fb, j], xht[:, j, :, :csz],
                        start=(j == 0), stop=False, perf_mode=PM.DoubleRow)
                for j in range(J):
                    nc.tensor.matmul(
                        ps[:], w1h_s[:, fb, j], xlt[:, j, :, :csz],
                        start=False, stop=False, perf_mode=PM.DoubleRow)
                for j in range(J):
                    nc.tensor.matmul(
                        ps[:], w1l_s[:, fb, j], xht[:, j, :, :csz],
                        start=False, stop=(j == J - 1), perf_mode=PM.DoubleRow)
                g32 = gpool.tile([P, CK], dt.float32, name="g32")[:, :csz]
                nc.scalar.activation(
                    g32[:], ps[:], AF.Gelu, bias=b1_s[:, fb:fb + 1], scale=1.0 / 256)
                h8v = h8[:, fb // 2, fb % 2, :csz]
                nc.scalar.activation(
                    h8v, ps[:], AF.Gelu, bias=b1_s[:, fb:fb + 1], scale=1.0 / 256)
                nc.vector.tensor_tensor(
                    hl[:, fb // 2, fb % 2, :csz], g32[:], h8v, ALU.subtract)

        def emit_mm2(c):
            csz = sizes[c]
            h8, hl = hs[c]
            for tt in range(csz // P):
                gt = c * 2 + tt
                t0 = tt * P
                for hb in range(HB):
                    ps2 = p2pool.tile([P, 256], dt.float32, name="p2")
                    for g in range(G):
                        nc.tensor.matmul(
                            ps2[:], h8[:, g, :, t0:t0 + P], w2h_s[:, hb, g],
                            start=(g == 0), stop=False, perf_mode=PM.DoubleRow)
                    for g in range(G):
                        nc.tensor.matmul(
                            ps2[:], hl[:, g, :, t0:t0 + P], w2h_s[:, hb, g],
                            start=False, stop=False, perf_mode=PM.DoubleRow)
                    for g in range(G):
                        nc.tensor.matmul(
                            ps2[:], h8[:, g, :, t0:t0 + P], w2l_s[:, hb, g],
                            start=False, stop=(g == G - 1), perf_mode=PM.DoubleRow)
                    ob = opool.tile([P, 256], dt.float32, name="ob")
                    nc.vector.tensor_tensor(
                        ob[:], ps2[:], b2w_s[:, hb * 256:(hb + 1) * 256], ALU.add)
                    nc.vector.tensor_scalar_mul(ob[:], ob[:], wdv_s[:, gt:gt + 1])
                    nc.sync.dma_start(
                        yc[gt * P:(gt + 1) * P, hb * 256:(hb + 1) * 256], ob[:])

        # Software pipeline: mm1 runs two chunks ahead of mm2 so the w2
        # stream has the whole first two mm1 phases to land.
        emit_mm1(0)
        if NC > 1:
            emit_mm1(1)
        for c in range(NC):
            emit_mm2(c)
            if c + 2 < NC:
                emit_mm1(c + 2)
    return nc


def _get_nc(C):
    if C not in _CACHE:
        nc = _build_nc(C)
        nc.finalize()
        _CACHE[C] = nc
    return _CACHE[C]


def _split8(a):
    hi = a.astype(fp8)
    lo = (a - hi.astype(np.float32)).astype(fp8)
    return hi, lo


def _x_layout(x8, idx, C):
    """[H, T] fp8 + token list -> [P, Cx*8] with [p, c, j, i, t] layout."""
    NC = (C + CK - 1) // CK
    Cx = NC * CK
    pad = np.zeros(Cx, dtype=np.int64)
    pad[:len(idx)] = idx
    g = x8[:, pad]                                   # [H, Cx]
    g = g.reshape(J, 2, P, NC, CK)                   # [j, i, p, c, t]
    return np.ascontiguousarray(
        g.transpose(2, 3, 0, 1, 4).reshape(P, Cx * 8))


def dispatch(hidden_states, router_w, router_b):
    """Host router: exact fp32 softmax top-2 + renormalized weights."""
    x = np.asarray(hidden_states, dtype=np.float32).reshape(T, H)
    logits = x @ np.asarray(router_w, dtype=np.float32)
    logits = logits + np.asarray(router_b, dtype=np.float32)
    part = np.argpartition(logits, E - 2, axis=1)[:, E - 2:]     # top-2 ids
    lg = np.take_along_axis(logits, part, axis=1)                # [T, 2]
    m = lg.max(axis=1, keepdims=True)
    e = np.exp(lg - m)
    wslot = e / e.sum(axis=1, keepdims=True)                     # [T, 2]
    idx_lists, wts = [], []
    for m_ in range(E):
        hit = part == m_
        rows = np.where(hit.any(axis=1))[0]
        idx_lists.append(rows)
        wts.append((wslot * hit)[rows].sum(axis=1))
    cmax = max(len(ix) for ix in idx_lists)
    C = max(P, ((cmax + P - 1) // P) * P)
    return x, idx_lists, wts, C


def make_in_maps(hidden_states, router_w, router_b, w1, b1, w2, b2):
    x, idx_lists, wts, C = dispatch(hidden_states, router_w, router_b)
    TTS = C // P
    xt = np.ascontiguousarray(x.T)                   # [H, T] f32
    x8h, x8l = _split8(xt)
    w1 = np.asarray(w1, dtype=np.float32)
    w2 = np.asarray(w2, dtype=np.float32)
    b1 = np.asarray(b1, dtype=np.float32)
    b2 = np.asarray(b2, dtype=np.float32)
    in_maps = []
    for m in range(E):
        ix = idx_lists[m]
        w1h, w1l = _split8(w1[m] * 256.0)            # [H, F]
        w2h, w2l = _split8(w2[m] * 256.0)            # [F, H]
        # [p, fb, j, i, f] = w1s[j*256+i*128+p, fb*128+f]
        w1m = [np.ascontiguousarray(
            a.reshape(J, 2, P, FB, P).transpose(2, 3, 0, 1, 4).reshape(P, -1))
            for a in (w1h, w1l)]
        # [p, hb, g, i, h] = w2s[(2g+i)*128+p, hb*256+h]
        w2m = [np.ascontiguousarray(
            a.reshape(G, 2, P, HB, 256).transpose(2, 3, 0, 1, 4).reshape(P, -1))
            for a in (w2h, w2l)]
        wcol = np.zeros(C, dtype=np.float32)
        wcol[:len(ix)] = wts[m] / 256.0
        in_maps.append({
            "xh": _x_layout(x8h, ix, C),
            "xl": _x_layout(x8l, ix, C),
            "w1h": w1m[0], "w1l": w1m[1],
            "w2h": w2m[0], "w2l": w2m[1],
            "b1d": np.ascontiguousarray(b1[m].reshape(FB, P).T),
            "b2w": np.ascontiguousarray(
                np.broadcast_to(b2[m] * 256.0, (P, H)).astype(np.float32)),
            "wdv": np.ascontiguousarray(wcol.reshape(TTS, P).T),
        })
    return in_maps, idx_lists, C


def run_device(in_maps, C):
    from concourse.bass_utils import run_bass_kernel_spmd

    nc = _get_nc(C)
    res = run_bass_kernel_spmd(nc, in_maps, core_ids=list(range(E)))
    return res.results


def kernel(hidden_states, router_w, router_b, w1, b1, w2, b2):
    in_maps, idx_lists, C = make_in_maps(
        hidden_states, router_w, router_b, w1, b1, w2, b2)
    # One retry guards against a rare transient execution glitch observed on
    # the very first load of a freshly compiled NEFF (garbage ~1e35 values);
    # a healthy output has absmax of a few units.
    last_err = None
    acc = None
    for attempt in range(3):
        try:
            results = run_device(in_maps, C)
        except Exception as e:  # transient NRT/axon failures observed
            last_err = e
            import time as _time
            _time.sleep(10)
            continue
        acc = np.zeros((T, H), dtype=np.float32)
        for m in range(E):
            ix = idx_lists[m]
            acc[ix] += np.asarray(results[m]["yc"], dtype=np.float32)[:len(ix)]
        if np.isfinite(acc).all() and np.abs(acc).max() < 1e4:
            return acc.reshape(B, S, H)
    if acc is None and last_err is not None:
        raise last_err
    return acc.reshape(B, S, H)
